# revision 2
# baseline (speedup 1.0000x reference)
"""Causal self-attention (B=4, T=2048, C=1024, H=16) on 8 TRN2 NeuronCores — v2.

Sharding: core c -> batch b = c//2, head-group g2 = c%2 (8 heads = 4 head-pairs
g, feature columns j0 = g2*512).  Host sums the two partial out-projections
per batch.  No collectives.

v2 dataflow (per core), engineered against the TimelineSim cost model:
  - QKV projections: fp8e4 DoubleRow matmuls (K=256/instr, 0.5 cyc/row),
    3-term hi/lo error compensation (xh@wh + xh@wl + xl@wh); weights
    pre-scaled by S=32 on host so fp8 stays in the normal range.
  - QK logits: fp32r (exact), d-major [128=2x64, T] q/k tiles as lhsT/rhs,
    diagonal narrowing, N>=256 kept for the fp32r full-rate rule.
  - exp on Act: scale=1/(S^2 sqrt(hd)) folded in, bf16 out.
  - causal staircase: gpsimd affine_select on bf16 diag tiles.
  - AV transposed: out y[q-tile, 65] with the attention weights stationary
    ([128,128] lhsT) and [v|ones] bf16 moving (65 rows/matmul); rowsums land
    in psum column 64.
  - normalize: reciprocal_approx_fast on the rowsum column + one bf16
    tensor_tensor multiply with a free-dim-broadcast scalar.
  - PE transpose (identity permutation) returns y to feature-major bf16.
  - out-projection: plain bf16 matmuls; 1/S scale folded into the out copy.
"""
import numpy as np

B, T, C = 4, 2048, 1024
NC = 8
P = 128
CO = 4           # 256-wide contraction chunks for QKV DoubleRow
QB = 512
NQB = T // QB    # 4
NKT = T // P     # 16
D = 64
W65 = 65
S = 32.0

_CACHE = {}

CFG = {"pp": 2, "attp": 2, "aep": 16, "xtp": 2, "ynp": 2, "obp": 4}


def _build():
    from contextlib import ExitStack
    import concourse.tile as tile
    from concourse import bacc, mybir

    F32 = mybir.dt.float32
    F32R = mybir.dt.float32r
    BF16 = mybir.dt.bfloat16
    F8E4 = mybir.dt.float8e4
    AF = mybir.ActivationFunctionType
    MUL = mybir.AluOpType.mult
    DR = mybir.MatmulPerfMode.DoubleRow
    ESCALE = 1.0 / (S * S * 8.0)

    nc = bacc.Bacc("TRN2", target_bir_lowering=False, debug=False,
                   dynamic_dma_scratch_size=2048)
    dram = {}
    for nm in ("xh", "xl"):
        dram[nm] = nc.dram_tensor(nm, [P, CO, 2, T], F8E4,
                                  kind="ExternalInput").ap()
    for nm in ("wqh", "wql", "wkh", "wkl", "wvh", "wvl"):
        dram[nm] = nc.dram_tensor(nm, [P, CO, 2, QB], F8E4,
                                  kind="ExternalInput").ap()
    dram["wpb"] = nc.dram_tensor("wpb", [P, 4, C], BF16,
                                 kind="ExternalInput").ap()
    dram["idm"] = nc.dram_tensor("idm", [P, P], BF16,
                                 kind="ExternalInput").ap()
    out = nc.dram_tensor("out", [T, C], F32, kind="ExternalOutput").ap()

    with tile.TileContext(nc) as tc, ExitStack() as ctx:
        persist = ctx.enter_context(tc.tile_pool(name="persist", bufs=1))
        qt = [persist.tile([P, T], F32R, tag=f"qt{g}", name=f"qt{g}")
              for g in range(4)]
        kt = [persist.tile([P, T], F32R, tag=f"kt{g}", name=f"kt{g}")
              for g in range(4)]
        # v natural: [kpos, ktile, head, 65] bf16, col 64 = ones
        vtp = persist.tile([P, NKT, 8, W65], BF16, tag="vtp", name="vtp")
        # y feature-major bf16: [feat(h2*64+d), g, t]
        yt = persist.tile([P, 4, T], BF16, tag="yt", name="yt")
        wts = {}
        for nm in ("wqh", "wql", "wkh", "wkl", "wvh", "wvl"):
            wts[nm] = persist.tile([P, CO, 2, QB], F8E4, tag=nm, name=nm)
        wpb = persist.tile([P, 4, C], BF16, tag="wpb", name="wpb")
        idt = persist.tile([P, P], BF16, tag="idt", name="idt")

        nc.vector.memset(vtp[:, :, :, D:W65], 1.0)
        nc.scalar.dma_start(idt[:], dram["idm"])

        with tc.tile_pool(name="xtp", bufs=CFG["xtp"]) as xtp, \
             tc.tile_pool(name="pp", bufs=CFG["pp"], space="PSUM") as pp, \
             tc.tile_pool(name="attp", bufs=CFG["attp"], space="PSUM") as attp, \
             tc.tile_pool(name="yap", bufs=1, space="PSUM") as yap, \
             tc.tile_pool(name="aep", bufs=CFG["aep"]) as aep, \
             tc.tile_pool(name="rcpp", bufs=2) as rcpp, \
             tc.tile_pool(name="ynp", bufs=CFG["ynp"]) as ynp, \
             tc.tile_pool(name="obp", bufs=CFG["obp"]) as obp:

            # ---- input DMAs: chunk-interleaved so the first DR group
            # (needs all of wqh/wql + xh/xl chunk ch) starts early ----
            xh0 = xtp.tile([P, CO, 2, QB], F8E4, tag="xh", name="xh0")
            xl0 = xtp.tile([P, CO, 2, QB], F8E4, tag="xl", name="xl0")
            for ch in range(CO):
                nc.sync.dma_start(wts["wqh"][:, ch], dram["wqh"][:, ch])
                nc.gpsimd.dma_start(xh0[:, ch], dram["xh"][:, ch, :, 0:QB])
                nc.sync.dma_start(wts["wql"][:, ch], dram["wql"][:, ch])
                nc.gpsimd.dma_start(xl0[:, ch], dram["xl"][:, ch, :, 0:QB])
            for nm in ("wkh", "wkl", "wvh", "wvl"):
                nc.scalar.dma_start(wts[nm][:], dram[nm])
            nc.scalar.dma_start(wpb[:], dram["wpb"])

            xts = {0: (xh0, xl0)}

            def load_x(tb):
                if tb in xts:
                    return xts[tb]
                xh_t = xtp.tile([P, CO, 2, QB], F8E4, tag="xh", name="xh")
                xl_t = xtp.tile([P, CO, 2, QB], F8E4, tag="xl", name="xl")
                nc.sync.dma_start(
                    xh_t[:], dram["xh"][:, :, :, tb * QB:(tb + 1) * QB])
                nc.sync.dma_start(
                    xl_t[:], dram["xl"][:, :, :, tb * QB:(tb + 1) * QB])
                xts[tb] = (xh_t, xl_t)
                return xts[tb]

            def emit_qk_tile(proj, g, tb):
                xh_t, xl_t = xts[tb]
                wh, wl = wts[f"w{proj}h"], wts[f"w{proj}l"]
                dst = qt if proj == "q" else kt
                ps = pp.tile([P, QB], F32, tag="pp", name="pp")
                n = 0
                for ch in range(CO):
                    for wt_, xt_ in ((wh, xh_t), (wl, xh_t), (wh, xl_t)):
                        nc.tensor.matmul(
                            ps[:], wt_[:, ch, :, g * P:(g + 1) * P],
                            xt_[:, ch], start=(n == 0),
                            stop=(n == 3 * CO - 1), perf_mode=DR)
                        n += 1
                nc.vector.tensor_copy(dst[g][:, tb * QB:(tb + 1) * QB], ps[:])

            def emit_v_tile(tt, tb):
                xh_t, xl_t = xts[tb]
                ki = tb * 4 + tt
                ps = pp.tile([P, QB], F32, tag="pp", name="pp")
                n = 0
                for ch in range(CO):
                    for wt_, xt_ in ((wts["wvh"], xh_t), (wts["wvl"], xh_t),
                                     (wts["wvh"], xl_t)):
                        nc.tensor.matmul(
                            ps[:], xt_[:, ch, :, tt * P:(tt + 1) * P],
                            wt_[:, ch], start=(n == 0),
                            stop=(n == 3 * CO - 1), perf_mode=DR)
                        n += 1
                nc.vector.tensor_copy(
                    vtp[:, ki, :, 0:D],
                    ps[:].rearrange("p (h d) -> p h d", d=D))

            def qkv_units(tb):
                return ([lambda p=p, g=g: emit_qk_tile(p, g, tb)
                         for p in ("q", "k") for g in range(4)]
                        + [lambda tt=tt: emit_v_tile(tt, tb) for tt in range(4)])

            def emit_op_tile(tt, mh):
                po = pp.tile([P, QB], F32, tag="pp", name="po")
                for g in range(4):
                    nc.tensor.matmul(
                        po[:], yt[:, g, tt * P:(tt + 1) * P],
                        wpb[:, g, mh * QB:(mh + 1) * QB],
                        start=(g == 0), stop=(g == 3))
                ob = obp.tile([P, QB], F32, tag="ob", name="ob")
                nc.vector.tensor_scalar(ob[:], po[:], 1.0 / S, None, MUL)
                nc.sync.dma_start(
                    out[tt * P:(tt + 1) * P, mh * QB:(mh + 1) * QB], ob[:])

            def op_units(qb):
                return [lambda tt=tt, mh=mh: emit_op_tile(tt, mh)
                        for tt in range(qb * 4, qb * 4 + 4) for mh in range(2)]



            # QKV for tb=0 runs unaccompanied (pipeline fill)
            for u in qkv_units(0):
                u()

            for qb in range(NQB):
                q0 = qb * QB
                # extra PE work to hide Act latency inside this phase
                extras = []
                if qb + 1 < NQB:
                    load_x(qb + 1)
                    extras += qkv_units(qb + 1)
                if qb > 0:
                    extras += op_units(qb - 1)
                if qb == NQB - 1:
                    extras += op_units(qb - 1) if False else []
                steps = 4 * (qb * 4 + 4)
                acc = 0.0
                per_step = len(extras) / steps

                def drip():
                    nonlocal acc
                    acc += per_step
                    while acc >= 1.0 and extras:
                        extras.pop(0)()
                        acc -= 1.0

                for g in range(4):
                    nk = qb * 4 + 4
                    aes = []
                    for ki in range(nk):
                        r = ki - qb * 4  # >=0 on diagonal tiles
                        d = r * P if r >= 0 else 0
                        dq = d if d in (P, 2 * P) else 0  # fp32r N>=256
                        ap = attp.tile([P, 2, QB], F32, tag="att", name="att")
                        for h2 in range(2):
                            rows = slice(h2 * D, h2 * D + D)
                            nc.tensor.matmul(
                                ap[:, h2, dq:QB],
                                kt[g][rows, ki * P:(ki + 1) * P],
                                qt[g][rows, q0 + dq:q0 + QB],
                                start=True, stop=True)
                        ae = aep.tile([P, 2, QB], BF16, tag="ae", name="ae")
                        nc.scalar.activation(ae[:, :, dq:QB], ap[:, :, dq:QB],
                                             AF.Exp, scale=ESCALE)
                        if r >= 0:
                            for h2 in range(2):
                                nc.gpsimd.affine_select(
                                    out=ae[:, h2, d:d + P],
                                    in_=ae[:, h2, d:d + P],
                                    compare_op=mybir.AluOpType.is_ge,
                                    fill=0.0, base=0,
                                    pattern=[[1, P]], channel_multiplier=-1)
                        aes.append(ae)
                        drip()
                    # AV per q-tile: sequential psum groups, immediate
                    # normalize, transposes pair-grouped in a borrowed
                    # att-pool tile (2KB zero-region discipline).
                    rcp = rcpp.tile([P, NQB, 2], F32, tag="rcp", name="rcp")
                    yn = ynp.tile([P, NQB, 2, D], BF16, tag="yn", name="yn")
                    trt = attp.tile([P, 2, QB], F32, tag="att", name="tr")
                    tr_bf = trt[:].bitcast(BF16)  # [P, 2, 1024]
                    for qi in range(NQB):
                        lastk = qb * 4 + qi
                        for h2 in range(2):
                            ya = yap.tile([P, QB], F32, tag=f"ya{h2}",
                                          name=f"ya{h2}")
                            for ki in range(lastk + 1):
                                r = ki - qb * 4
                                nc.tensor.matmul(
                                    ya[:, 0:W65],
                                    aes[ki][:, h2, qi * P:(qi + 1) * P],
                                    vtp[:, ki, 2 * g + h2, :],
                                    start=(ki == 0), stop=(ki == lastk))
                            nc.vector.reciprocal_approx_fast(
                                rcp[:, qi, h2:h2 + 1], ya[:, D:D + 1])
                            nc.vector.tensor_tensor(
                                yn[:, qi, h2, :], ya[:, 0:D],
                                rcp[:, qi, h2:h2 + 1].broadcast_to([P, D]),
                                MUL)
                        trp = tr_bf[:, qi // 2, (qi % 2) * QB:(qi % 2) * QB + P]
                        nc.tensor.matmul(trp, yn[:, qi], idt[:],
                                         is_transpose=True,
                                         start=(qi % 2 == 0),
                                         stop=(qi % 2 == 1))
                        if qi % 2 == 1:
                            for q2 in (qi - 1, qi):
                                t2 = tr_bf[:, q2 // 2,
                                           (q2 % 2) * QB:(q2 % 2) * QB + P]
                                nc.vector.tensor_copy(
                                    yt[:, g,
                                       (qb * 4 + q2) * P:(qb * 4 + q2 + 1) * P],
                                    t2)
                        drip()
                while extras:
                    extras.pop(0)()
            # final out-projection block
            for u in op_units(NQB - 1):
                u()

    nc.finalize()
    return nc


def _prep_inputs(x, Wq, Wk, Wv, Wp):
    import ml_dtypes
    F8 = ml_dtypes.float8_e4m3fn
    BF = ml_dtypes.bfloat16
    f32 = np.float32

    def dr_layout(a):  # [1024, N] -> [128, 4, 2, N] (ci, ch, j)
        n = a.shape[1]
        return np.ascontiguousarray(
            a.reshape(CO, 2, P, n).transpose(2, 0, 1, 3))

    def hilo8(a):
        h = np.clip(a, -448, 448).astype(F8)
        l = np.clip(a - h.astype(f32), -448, 448).astype(F8)
        return dr_layout(h), dr_layout(l)

    idm = np.eye(P, dtype=np.float32).astype(BF)
    in_maps = []
    for c in range(NC):
        b, g2 = c // 2, c % 2
        j0 = g2 * 512
        xh, xl = hilo8(x[b].T.astype(f32))
        wqh, wql = hilo8((Wq[j0:j0 + 512] * S).T.astype(f32))
        wkh, wkl = hilo8((Wk[j0:j0 + 512] * S).T.astype(f32))
        wvh, wvl = hilo8((Wv[j0:j0 + 512] * S).T.astype(f32))
        wpb = np.ascontiguousarray(
            Wp[:, j0:j0 + 512].T.astype(f32).reshape(4, P, C)
            .transpose(1, 0, 2)).astype(BF)          # [128, 4, 1024]
        in_maps.append({
            "xh": xh, "xl": xl,
            "wqh": wqh, "wql": wql,
            "wkh": wkh, "wkl": wkl,
            "wvh": wvh, "wvl": wvl,
            "wpb": wpb,
            "idm": idm,
        })
    return in_maps


def kernel(x, Wq, Wk, Wv, Wp, _trace=False):
    from concourse.bass_utils import run_bass_kernel_spmd

    x = np.asarray(x); Wq = np.asarray(Wq); Wk = np.asarray(Wk)
    Wv = np.asarray(Wv); Wp = np.asarray(Wp)

    if "nc" not in _CACHE:
        _CACHE["nc"] = _build()
    nc = _CACHE["nc"]

    in_maps = _prep_inputs(x, Wq, Wk, Wv, Wp)
    res = run_bass_kernel_spmd(nc, in_maps, core_ids=list(range(NC)),
                               trace=_trace)
    outs = [r["out"] for r in res.results]
    full = np.empty((B, T, C), np.float32)
    for b in range(B):
        full[b] = outs[2 * b] + outs[2 * b + 1]
    if _trace:
        _CACHE["last_results"] = res
    return full


# revision 3
# speedup vs baseline: 1.0942x; 1.0942x over previous
"""Causal self-attention (B=4, T=2048, C=1024, H=16) on 8 TRN2 NeuronCores — v2.

Sharding: core c -> batch b = c//2, head-group g2 = c%2 (8 heads = 4 head-pairs
g, feature columns j0 = g2*512).  Host sums the two partial out-projections
per batch.  No collectives.

Dataflow (per core), engineered against the TimelineSim cost model:
  - QKV projections: fp8e4 DoubleRow matmuls (K=256/instr, 0.5 cyc/row),
    3-term hi/lo error compensation (xh@wh + xh@wl + xl@wh); weights
    pre-scaled by S=32 on host so fp8 stays in the normal range.
  - QK logits: bf16 q/k (1 cyc/row at any N -> full diagonal narrowing).
  - exp on Act: scale=1/(S^2 sqrt(hd)) folded in, bf16 out.
  - causal staircase: gpsimd affine_select on bf16 diag tiles.
  - AV transposed: out y[q-tile, 65] with the attention weights stationary
    ([128,128] lhsT) and [v|ones] bf16 moving (65 rows/matmul); rowsums land
    in psum column 64.  One accumulation group per psum bank at a time
    (2KB zero-region discipline), normalize immediately per q-tile.
  - y back to feature-major via DMA xbar transpose (no PE/DVE involved).
  - out-projection: plain bf16 matmuls; 1/S scale folded into the out copy.
  - emission interleaving: each block's AV/normalize tail and the next
    t-block's QKV tiles drip into the Act-paced QK/exp stream so neither
    PE nor Act ever drains.
"""
import numpy as np

B, T, C = 4, 2048, 1024
NC = 8
P = 128
CO = 4           # 256-wide contraction chunks for QKV DoubleRow
QB = 512
NQB = T // QB    # 4
NKT = T // P     # 16
D = 64
W65 = 65
S = 32.0

_CACHE = {}

CFG = {"pp": 2, "attp": 2, "aep": 40, "xtp": 3, "ynp": 3, "rcpp": 3, "obp": 6}


def _build():
    from contextlib import ExitStack
    import concourse.tile as tile
    from concourse import bacc, mybir

    F32 = mybir.dt.float32
    BF16 = mybir.dt.bfloat16
    F8E4 = mybir.dt.float8e4
    AF = mybir.ActivationFunctionType
    MUL = mybir.AluOpType.mult
    DR = mybir.MatmulPerfMode.DoubleRow
    ESCALE = 1.0 / (S * S * 8.0)

    nc = bacc.Bacc("TRN2", target_bir_lowering=False, debug=False,
                   dynamic_dma_scratch_size=2048)
    dram = {}
    for nm in ("xh", "xl"):
        dram[nm] = nc.dram_tensor(nm, [P, CO, 2, T], F8E4,
                                  kind="ExternalInput").ap()
    for nm in ("wqh", "wql", "wkh", "wkl", "wvh", "wvl"):
        dram[nm] = nc.dram_tensor(nm, [P, CO, 2, QB], F8E4,
                                  kind="ExternalInput").ap()
    dram["wpb"] = nc.dram_tensor("wpb", [P, 4, C], BF16,
                                 kind="ExternalInput").ap()
    out = nc.dram_tensor("out", [T, C], F32, kind="ExternalOutput").ap()

    with tile.TileContext(nc) as tc, ExitStack() as ctx:
        persist = ctx.enter_context(tc.tile_pool(name="persist", bufs=1))
        qt = [persist.tile([P, T], BF16, tag=f"qt{g}", name=f"qt{g}")
              for g in range(4)]
        kt = [persist.tile([P, T], BF16, tag=f"kt{g}", name=f"kt{g}")
              for g in range(4)]
        # v natural: [kpos, ktile, head, 65] bf16, col 64 = ones
        vtp = persist.tile([P, NKT, 8, W65], BF16, tag="vtp", name="vtp")
        # y feature-major bf16, per q-super-block: [feat(h2*64+d), g, 512]
        yt = [persist.tile([P, 4, QB], BF16, tag=f"yt{qb}", name=f"yt{qb}")
              for qb in range(NQB)]
        wts = {}
        for nm in ("wqh", "wql", "wkh", "wkl", "wvh", "wvl"):
            wts[nm] = persist.tile([P, CO, 2, QB], F8E4, tag=nm, name=nm)
        wpb = persist.tile([P, 4, C], BF16, tag="wpb", name="wpb")

        nc.vector.memset(vtp[:, :, :, D:W65], 1.0)

        with tc.tile_pool(name="xtp", bufs=CFG["xtp"]) as xtp, \
             tc.tile_pool(name="pp", bufs=CFG["pp"], space="PSUM") as pp, \
             tc.tile_pool(name="attp", bufs=CFG["attp"], space="PSUM") as attp, \
             tc.tile_pool(name="yap", bufs=1, space="PSUM") as yap, \
             tc.tile_pool(name="aep", bufs=CFG["aep"]) as aep, \
             tc.tile_pool(name="rcpp", bufs=CFG["rcpp"]) as rcpp, \
             tc.tile_pool(name="ynp", bufs=CFG["ynp"]) as ynp, \
             tc.tile_pool(name="obp", bufs=CFG["obp"]) as obp:

            # ---- input DMAs: whole tensors, 3 queues, first-needed first
            xh0 = xtp.tile([P, CO, 2, QB], F8E4, tag="xh", name="xh0")
            xl0 = xtp.tile([P, CO, 2, QB], F8E4, tag="xl", name="xl0")
            nc.sync.dma_start(wts["wqh"][:], dram["wqh"])
            nc.gpsimd.dma_start(xh0[:], dram["xh"][:, :, :, 0:QB])
            nc.sync.dma_start(wts["wql"][:], dram["wql"])
            nc.gpsimd.dma_start(xl0[:], dram["xl"][:, :, :, 0:QB])
            nc.sync.dma_start(wts["wkh"][:], dram["wkh"])
            nc.sync.dma_start(wts["wkl"][:], dram["wkl"])
            for nm in ("wvh", "wvl"):
                nc.scalar.dma_start(wts[nm][:], dram[nm])
            nc.scalar.dma_start(wpb[:], dram["wpb"])

            xts = {0: (xh0, xl0)}

            def load_x(tb):
                if tb in xts:
                    return xts[tb]
                xh_t = xtp.tile([P, CO, 2, QB], F8E4, tag="xh", name="xh")
                xl_t = xtp.tile([P, CO, 2, QB], F8E4, tag="xl", name="xl")
                nc.sync.dma_start(
                    xh_t[:], dram["xh"][:, :, :, tb * QB:(tb + 1) * QB])
                nc.sync.dma_start(
                    xl_t[:], dram["xl"][:, :, :, tb * QB:(tb + 1) * QB])
                xts[tb] = (xh_t, xl_t)
                return xts[tb]

            def emit_qk_tile(proj, g, tb):
                xh_t, xl_t = xts[tb]
                wh, wl = wts[f"w{proj}h"], wts[f"w{proj}l"]
                dst = qt if proj == "q" else kt
                ps = pp.tile([P, QB], F32, tag="pp", name="pp")
                n = 0
                for ch in range(CO):
                    for wt_, xt_ in ((wh, xh_t), (wl, xh_t), (wh, xl_t)):
                        nc.tensor.matmul(
                            ps[:], wt_[:, ch, :, g * P:(g + 1) * P],
                            xt_[:, ch], start=(n == 0),
                            stop=(n == 3 * CO - 1), perf_mode=DR)
                        n += 1
                        if n == 6:
                            yield
                nc.vector.tensor_copy(dst[g][:, tb * QB:(tb + 1) * QB], ps[:])

            def emit_v_tile(tt, tb):
                xh_t, xl_t = xts[tb]
                ki = tb * 4 + tt
                ps = pp.tile([P, QB], F32, tag="pp", name="pp")
                n = 0
                for ch in range(CO):
                    for wt_, xt_ in ((wts["wvh"], xh_t), (wts["wvl"], xh_t),
                                     (wts["wvh"], xl_t)):
                        nc.tensor.matmul(
                            ps[:], xt_[:, ch, :, tt * P:(tt + 1) * P],
                            wt_[:, ch], start=(n == 0),
                            stop=(n == 3 * CO - 1), perf_mode=DR)
                        n += 1
                        if n == 6:
                            yield
                nc.vector.tensor_copy(
                    vtp[:, ki, :, 0:D],
                    ps[:].rearrange("p (h d) -> p h d", d=D))

            def emit_op_tile(tt, mh):
                po = yap.tile([P, QB], F32, tag=f"ya{(tt * 2 + mh) % 2}",
                              name="po")
                for g in range(4):
                    nc.tensor.matmul(
                        po[:], yt[tt // 4][:, g, (tt % 4) * P:(tt % 4 + 1) * P],
                        wpb[:, g, mh * QB:(mh + 1) * QB],
                        start=(g == 0), stop=(g == 3))
                    if g == 1:
                        yield
                ob = obp.tile([P, QB], F32, tag="ob", name="ob")
                nc.vector.tensor_scalar(ob[:], po[:], 1.0 / S, None, MUL)
                nc.sync.dma_start(
                    out[tt * P:(tt + 1) * P, mh * QB:(mh + 1) * QB], ob[:])

            def _unit(fn, pieces, *a):
                def mk():
                    return fn(*a)
                mk.pieces = pieces
                return mk

            def qkv_units(tb):
                return ([_unit(emit_qk_tile, 2, "q", g, tb) for g in range(4)]
                        + [_unit(emit_v_tile, 2, tt, tb) for tt in range(4)]
                        + [_unit(emit_qk_tile, 2, "k", g, tb) for g in range(4)])

            def op_units(qb):
                return [_unit(emit_op_tile, 2, tt, mh)
                        for tt in range(qb * 4, qb * 4 + 4) for mh in range(2)]

            def block_tail(g, qb, aes):
                """AV sweeps + normalize + DMA-transpose for one block."""
                rcp = rcpp.tile([P, NQB, 2], F32, tag="rcp", name="rcp")
                yn = ynp.tile([P, NQB, 2, D], BF16, tag="yn", name="yn")
                for qi in range(NQB):
                    lastk = qb * 4 + qi
                    for h2 in range(2):
                        ya = yap.tile([P, QB], F32, tag=f"ya{h2}",
                                      name=f"ya{h2}")
                        for ki in range(lastk + 1):
                            nc.tensor.matmul(
                                ya[:, 0:W65],
                                aes[ki][:, h2, qi * P:(qi + 1) * P],
                                vtp[:, ki, 2 * g + h2, :],
                                start=(ki == 0), stop=(ki == lastk))
                        nc.vector.reciprocal_approx_fast(
                            rcp[:, qi, h2:h2 + 1], ya[:, D:D + 1])
                        nc.vector.tensor_tensor(
                            yn[:, qi, h2, :], ya[:, 0:D],
                            rcp[:, qi, h2:h2 + 1].broadcast_to([P, D]), MUL)
                        yield
                    nc.sync.dma_start_transpose(
                        yt[qb][:, g, qi * P:(qi + 1) * P], yn[:, qi])

            tail_gens = []

            def advance_tail():
                while tail_gens:
                    try:
                        next(tail_gens[0])
                        return True
                    except StopIteration:
                        tail_gens.pop(0)
                return False

            # QKV for tb=0 runs unaccompanied (pipeline fill)
            for u in qkv_units(0):
                for _ in u():
                    pass

            for qb in range(NQB):
                q0 = qb * QB
                extras = []
                if qb + 1 < NQB:
                    load_x(qb + 1)
                    extras += qkv_units(qb + 1)
                if qb == 2:
                    extras += op_units(0)
                elif qb == 3:
                    extras += op_units(1) + op_units(2)
                gens = [u() for u in extras]
                steps = 4 * (qb * 4 + 4)
                acc = 0.0
                per_step = sum(u.pieces for u in extras) / steps

                def drip():
                    nonlocal acc
                    acc += per_step
                    while acc >= 1.0 and gens:
                        try:
                            next(gens[0])
                        except StopIteration:
                            gens.pop(0)
                        else:
                            acc -= 1.0

                for g in range(4):
                    nk = qb * 4 + 4
                    aes = []
                    for ki in range(nk):
                        r = ki - qb * 4  # >=0 on diagonal tiles
                        dq = r * P if r >= 0 else 0
                        ap = attp.tile([P, 2, QB], F32, tag="att", name="att")
                        for h2 in range(2):
                            rows = slice(h2 * D, h2 * D + D)
                            nc.tensor.matmul(
                                ap[:, h2, dq:QB],
                                kt[g][rows, ki * P:(ki + 1) * P],
                                qt[g][rows, q0 + dq:q0 + QB],
                                start=True, stop=True)
                        ae = aep.tile([P, 2, QB], BF16, tag="ae", name="ae")
                        nc.scalar.activation(ae[:, :, dq:QB], ap[:, :, dq:QB],
                                             AF.Exp, scale=ESCALE)
                        if r >= 0:
                            for h2 in range(2):
                                nc.gpsimd.affine_select(
                                    out=ae[:, h2, dq:dq + P],
                                    in_=ae[:, h2, dq:dq + P],
                                    compare_op=mybir.AluOpType.is_ge,
                                    fill=0.0, base=0,
                                    pattern=[[1, P]], channel_multiplier=-1)
                        aes.append(ae)
                        advance_tail()
                        advance_tail()
                        drip()
                    tail_gens.append(block_tail(g, qb, aes))
                while gens:
                    try:
                        next(gens[0])
                    except StopIteration:
                        gens.pop(0)
            while advance_tail():
                pass
            # final out-projection block
            for u in op_units(NQB - 1):
                for _ in u():
                    pass

    nc.finalize()
    return nc


def _prep_inputs(x, Wq, Wk, Wv, Wp):
    import ml_dtypes
    F8 = ml_dtypes.float8_e4m3fn
    BF = ml_dtypes.bfloat16
    f32 = np.float32

    def dr_layout(a):  # [1024, N] -> [128, 4, 2, N] (ci, ch, j)
        n = a.shape[1]
        return np.ascontiguousarray(
            a.reshape(CO, 2, P, n).transpose(2, 0, 1, 3))

    def hilo8(a):
        h = np.clip(a, -448, 448).astype(F8)
        l = np.clip(a - h.astype(f32), -448, 448).astype(F8)
        return dr_layout(h), dr_layout(l)

    in_maps = []
    for c in range(NC):
        b, g2 = c // 2, c % 2
        j0 = g2 * 512
        xh, xl = hilo8(x[b].T.astype(f32))
        wqh, wql = hilo8((Wq[j0:j0 + 512] * S).T.astype(f32))
        wkh, wkl = hilo8((Wk[j0:j0 + 512] * S).T.astype(f32))
        wvh, wvl = hilo8((Wv[j0:j0 + 512] * S).T.astype(f32))
        wpb = np.ascontiguousarray(
            Wp[:, j0:j0 + 512].T.astype(f32).reshape(4, P, C)
            .transpose(1, 0, 2)).astype(BF)          # [128, 4, 1024]
        in_maps.append({
            "xh": xh, "xl": xl,
            "wqh": wqh, "wql": wql,
            "wkh": wkh, "wkl": wkl,
            "wvh": wvh, "wvl": wvl,
            "wpb": wpb,
        })
    return in_maps


def kernel(x, Wq, Wk, Wv, Wp, _trace=False):
    from concourse.bass_utils import run_bass_kernel_spmd

    x = np.asarray(x); Wq = np.asarray(Wq); Wk = np.asarray(Wk)
    Wv = np.asarray(Wv); Wp = np.asarray(Wp)

    if "nc" not in _CACHE:
        _CACHE["nc"] = _build()
    nc = _CACHE["nc"]

    in_maps = _prep_inputs(x, Wq, Wk, Wv, Wp)
    res = run_bass_kernel_spmd(nc, in_maps, core_ids=list(range(NC)),
                               trace=_trace)
    outs = [r["out"] for r in res.results]
    full = np.empty((B, T, C), np.float32)
    for b in range(B):
        full[b] = outs[2 * b] + outs[2 * b + 1]
    if _trace:
        _CACHE["last_results"] = res
    return full


# revision 4
# speedup vs baseline: 1.1023x; 1.0074x over previous
"""Causal self-attention (B=4, T=2048, C=1024, H=16) on 8 TRN2 NeuronCores — v2.

Sharding: core c -> batch b = c//2, head-group g2 = c%2 (8 heads = 4 head-pairs
g, feature columns j0 = g2*512).  Host sums the two partial out-projections
per batch.  No collectives.

Dataflow (per core), engineered against the TimelineSim cost model:
  - QKV projections: fp8e4 DoubleRow matmuls (K=256/instr, 0.5 cyc/row),
    3-term hi/lo error compensation (xh@wh + xh@wl + xl@wh); weights
    pre-scaled by S=32 on host so fp8 stays in the normal range.
  - QK logits: bf16 q/k (1 cyc/row at any N -> full diagonal narrowing).
  - exp on Act: scale=1/(S^2 sqrt(hd)) folded in, bf16 out.
  - causal staircase: gpsimd affine_select on bf16 diag tiles.
  - AV transposed: out y[q-tile, 65] with the attention weights stationary
    ([128,128] lhsT) and [v|ones] bf16 moving (65 rows/matmul); rowsums land
    in psum column 64.  One accumulation group per psum bank at a time
    (2KB zero-region discipline), normalize immediately per q-tile.
  - y back to feature-major via DMA xbar transpose (no PE/DVE involved).
  - out-projection: plain bf16 matmuls; 1/S scale folded into the out copy.
  - emission interleaving: each block's AV/normalize tail and the next
    t-block's QKV tiles drip into the Act-paced QK/exp stream so neither
    PE nor Act ever drains.
"""
import numpy as np

B, T, C = 4, 2048, 1024
NC = 8
P = 128
CO = 4           # 256-wide contraction chunks for QKV DoubleRow
QB = 512
NQB = T // QB    # 4
NKT = T // P     # 16
D = 64
W65 = 65
S = 32.0

_CACHE = {}

CFG = {"pp": 2, "attp": 2, "aep": 40, "xtp": 3, "ynp": 3, "rcpp": 3, "obp": 6}


def _build():
    from contextlib import ExitStack
    import concourse.tile as tile
    from concourse import bacc, mybir

    F32 = mybir.dt.float32
    BF16 = mybir.dt.bfloat16
    F8E4 = mybir.dt.float8e4
    AF = mybir.ActivationFunctionType
    MUL = mybir.AluOpType.mult
    DR = mybir.MatmulPerfMode.DoubleRow
    ESCALE = 1.0 / (S * S * 8.0)

    nc = bacc.Bacc("TRN2", target_bir_lowering=False, debug=False,
                   dynamic_dma_scratch_size=2048)
    dram = {}
    for nm in ("xh", "xl"):
        dram[nm] = nc.dram_tensor(nm, [P, CO, 2, T], F8E4,
                                  kind="ExternalInput").ap()
    for nm in ("wqh", "wql", "wkh", "wkl", "wvh", "wvl"):
        dram[nm] = nc.dram_tensor(nm, [P, CO, 2, QB], F8E4,
                                  kind="ExternalInput").ap()
    dram["wpb"] = nc.dram_tensor("wpb", [P, 4, C], BF16,
                                 kind="ExternalInput").ap()
    out = nc.dram_tensor("out", [T, C], F32, kind="ExternalOutput").ap()

    with tile.TileContext(nc) as tc, ExitStack() as ctx:
        persist = ctx.enter_context(tc.tile_pool(name="persist", bufs=1))
        qt = [persist.tile([P, T], BF16, tag=f"qt{g}", name=f"qt{g}")
              for g in range(4)]
        kt = [persist.tile([P, T], BF16, tag=f"kt{g}", name=f"kt{g}")
              for g in range(4)]
        # v natural: [kpos, ktile, head, 65] bf16, col 64 = ones
        vtp = persist.tile([P, NKT, 8, W65], BF16, tag="vtp", name="vtp")
        # y feature-major bf16, per q-super-block: [feat(h2*64+d), g, 512]
        yt = [persist.tile([P, 4, QB], BF16, tag=f"yt{qb}", name=f"yt{qb}")
              for qb in range(NQB)]
        wts = {}
        for nm in ("wqh", "wql", "wkh", "wkl", "wvh", "wvl"):
            wts[nm] = persist.tile([P, CO, 2, QB], F8E4, tag=nm, name=nm)
        wpb = persist.tile([P, 4, C], BF16, tag="wpb", name="wpb")

        nc.vector.memset(vtp[:, :, :, D:W65], 1.0)

        with tc.tile_pool(name="xtp", bufs=CFG["xtp"]) as xtp, \
             tc.tile_pool(name="pp", bufs=CFG["pp"], space="PSUM") as pp, \
             tc.tile_pool(name="attp", bufs=CFG["attp"], space="PSUM") as attp, \
             tc.tile_pool(name="yap", bufs=1, space="PSUM") as yap, \
             tc.tile_pool(name="aep", bufs=CFG["aep"]) as aep, \
             tc.tile_pool(name="rcpp", bufs=CFG["rcpp"]) as rcpp, \
             tc.tile_pool(name="ynp", bufs=CFG["ynp"]) as ynp, \
             tc.tile_pool(name="obp", bufs=CFG["obp"]) as obp:

            # ---- input DMAs: whole tensors, 3 queues, first-needed first
            xh0 = xtp.tile([P, CO, 2, QB], F8E4, tag="xh", name="xh0")
            xl0 = xtp.tile([P, CO, 2, QB], F8E4, tag="xl", name="xl0")
            nc.sync.dma_start(wts["wqh"][:], dram["wqh"])
            nc.gpsimd.dma_start(xh0[:], dram["xh"][:, :, :, 0:QB])
            nc.sync.dma_start(wts["wql"][:], dram["wql"])
            nc.gpsimd.dma_start(xl0[:], dram["xl"][:, :, :, 0:QB])
            nc.sync.dma_start(wts["wkh"][:], dram["wkh"])
            nc.sync.dma_start(wts["wkl"][:], dram["wkl"])
            for nm in ("wvh", "wvl"):
                nc.scalar.dma_start(wts[nm][:], dram[nm])
            nc.scalar.dma_start(wpb[:], dram["wpb"])

            xts = {0: (xh0, xl0)}

            def load_x(tb):
                if tb in xts:
                    return xts[tb]
                xh_t = xtp.tile([P, CO, 2, QB], F8E4, tag="xh", name="xh")
                xl_t = xtp.tile([P, CO, 2, QB], F8E4, tag="xl", name="xl")
                nc.sync.dma_start(
                    xh_t[:], dram["xh"][:, :, :, tb * QB:(tb + 1) * QB])
                nc.sync.dma_start(
                    xl_t[:], dram["xl"][:, :, :, tb * QB:(tb + 1) * QB])
                xts[tb] = (xh_t, xl_t)
                return xts[tb]

            def emit_qk_tile(proj, g, tb):
                xh_t, xl_t = xts[tb]
                wh, wl = wts[f"w{proj}h"], wts[f"w{proj}l"]
                dst = qt if proj == "q" else kt
                ps = pp.tile([P, QB], F32, tag="pp", name="pp")
                n = 0
                for wt_, xt_ in ((wh, xh_t), (wl, xh_t), (wh, xl_t)):
                    for ch in range(CO):
                        nc.tensor.matmul(
                            ps[:], wt_[:, ch, :, g * P:(g + 1) * P],
                            xt_[:, ch], start=(n == 0),
                            stop=(n == 3 * CO - 1), perf_mode=DR)
                        n += 1
                        if n == 6:
                            yield
                nc.vector.tensor_copy(dst[g][:, tb * QB:(tb + 1) * QB], ps[:])

            def emit_v_tile(tt, tb):
                xh_t, xl_t = xts[tb]
                ki = tb * 4 + tt
                ps = pp.tile([P, QB], F32, tag="pp", name="pp")
                n = 0
                for wt_, xt_ in ((wts["wvh"], xh_t), (wts["wvl"], xh_t),
                                 (wts["wvh"], xl_t)):
                    for ch in range(CO):
                        nc.tensor.matmul(
                            ps[:], xt_[:, ch, :, tt * P:(tt + 1) * P],
                            wt_[:, ch], start=(n == 0),
                            stop=(n == 3 * CO - 1), perf_mode=DR)
                        n += 1
                        if n == 6:
                            yield
                nc.vector.tensor_copy(
                    vtp[:, ki, :, 0:D],
                    ps[:].rearrange("p (h d) -> p h d", d=D))

            def emit_op_tile(tt, mh):
                po = yap.tile([P, QB], F32, tag=f"ya{(tt * 2 + mh) % 2}",
                              name="po")
                for g in range(4):
                    nc.tensor.matmul(
                        po[:], yt[tt // 4][:, g, (tt % 4) * P:(tt % 4 + 1) * P],
                        wpb[:, g, mh * QB:(mh + 1) * QB],
                        start=(g == 0), stop=(g == 3))
                    if g == 1:
                        yield
                ob = obp.tile([P, QB], F32, tag="ob", name="ob")
                nc.vector.tensor_scalar(ob[:], po[:], 1.0 / S, None, MUL)
                nc.sync.dma_start(
                    out[tt * P:(tt + 1) * P, mh * QB:(mh + 1) * QB], ob[:])

            def _unit(fn, pieces, *a):
                def mk():
                    return fn(*a)
                mk.pieces = pieces
                return mk

            def qkv_units(tb):
                return ([_unit(emit_qk_tile, 2, "q", g, tb) for g in range(4)]
                        + [_unit(emit_v_tile, 2, tt, tb) for tt in range(4)]
                        + [_unit(emit_qk_tile, 2, "k", g, tb) for g in range(4)])

            def op_units(qb):
                return [_unit(emit_op_tile, 2, tt, mh)
                        for tt in range(qb * 4, qb * 4 + 4) for mh in range(2)]

            def block_tail(g, qb, aes):
                """AV sweeps + normalize + DMA-transpose for one block."""
                rcp = rcpp.tile([P, NQB, 2], F32, tag="rcp", name="rcp")
                yn = ynp.tile([P, NQB, 2, D], BF16, tag="yn", name="yn")
                for qi in range(NQB):
                    lastk = qb * 4 + qi
                    for h2 in range(2):
                        ya = yap.tile([P, QB], F32, tag=f"ya{h2}",
                                      name=f"ya{h2}")
                        for ki in range(lastk + 1):
                            nc.tensor.matmul(
                                ya[:, 0:W65],
                                aes[ki][:, h2, qi * P:(qi + 1) * P],
                                vtp[:, ki, 2 * g + h2, :],
                                start=(ki == 0), stop=(ki == lastk))
                        nc.vector.reciprocal_approx_fast(
                            rcp[:, qi, h2:h2 + 1], ya[:, D:D + 1])
                        nc.vector.tensor_tensor(
                            yn[:, qi, h2, :], ya[:, 0:D],
                            rcp[:, qi, h2:h2 + 1].broadcast_to([P, D]), MUL)
                        yield
                    nc.sync.dma_start_transpose(
                        yt[qb][:, g, qi * P:(qi + 1) * P], yn[:, qi])

            tail_gens = []

            def advance_tail():
                while tail_gens:
                    try:
                        next(tail_gens[0])
                        return True
                    except StopIteration:
                        tail_gens.pop(0)
                return False

            # QKV for tb=0 runs unaccompanied (pipeline fill)
            for u in qkv_units(0):
                for _ in u():
                    pass

            for qb in range(NQB):
                q0 = qb * QB
                extras = []
                if qb + 1 < NQB:
                    load_x(qb + 1)
                    extras += qkv_units(qb + 1)
                if qb == 2:
                    extras += op_units(0)
                elif qb == 3:
                    extras += op_units(1) + op_units(2)
                gens = [u() for u in extras]
                steps = 4 * (qb * 4 + 4)
                acc = 0.0
                per_step = sum(u.pieces for u in extras) / steps

                def drip():
                    nonlocal acc
                    acc += per_step
                    while acc >= 1.0 and gens:
                        try:
                            next(gens[0])
                        except StopIteration:
                            gens.pop(0)
                        else:
                            acc -= 1.0

                for g in range(4):
                    nk = qb * 4 + 4
                    aes = []
                    for ki in range(nk):
                        r = ki - qb * 4  # >=0 on diagonal tiles
                        dq = r * P if r >= 0 else 0
                        ap = attp.tile([P, 2, QB], F32, tag="att", name="att")
                        for h2 in range(2):
                            rows = slice(h2 * D, h2 * D + D)
                            nc.tensor.matmul(
                                ap[:, h2, dq:QB],
                                kt[g][rows, ki * P:(ki + 1) * P],
                                qt[g][rows, q0 + dq:q0 + QB],
                                start=True, stop=True)
                        ae = aep.tile([P, 2, QB], BF16, tag="ae", name="ae")
                        nc.scalar.activation(ae[:, :, dq:QB], ap[:, :, dq:QB],
                                             AF.Exp, scale=ESCALE)
                        if r >= 0:
                            for h2 in range(2):
                                nc.gpsimd.affine_select(
                                    out=ae[:, h2, dq:dq + P],
                                    in_=ae[:, h2, dq:dq + P],
                                    compare_op=mybir.AluOpType.is_ge,
                                    fill=0.0, base=0,
                                    pattern=[[1, P]], channel_multiplier=-1)
                        aes.append(ae)
                        advance_tail()
                        advance_tail()
                        drip()
                    tail_gens.append(block_tail(g, qb, aes))
                while gens:
                    try:
                        next(gens[0])
                    except StopIteration:
                        gens.pop(0)
            while advance_tail():
                pass
            # final out-projection block
            for u in op_units(NQB - 1):
                for _ in u():
                    pass

    nc.finalize()
    return nc


def _prep_inputs(x, Wq, Wk, Wv, Wp):
    import ml_dtypes
    F8 = ml_dtypes.float8_e4m3fn
    BF = ml_dtypes.bfloat16
    f32 = np.float32

    def dr_layout(a):  # [1024, N] -> [128, 4, 2, N] (ci, ch, j)
        n = a.shape[1]
        return np.ascontiguousarray(
            a.reshape(CO, 2, P, n).transpose(2, 0, 1, 3))

    def hilo8(a):
        h = np.clip(a, -448, 448).astype(F8)
        l = np.clip(a - h.astype(f32), -448, 448).astype(F8)
        return dr_layout(h), dr_layout(l)

    in_maps = []
    for c in range(NC):
        b, g2 = c // 2, c % 2
        j0 = g2 * 512
        xh, xl = hilo8(x[b].T.astype(f32))
        wqh, wql = hilo8((Wq[j0:j0 + 512] * S).T.astype(f32))
        wkh, wkl = hilo8((Wk[j0:j0 + 512] * S).T.astype(f32))
        wvh, wvl = hilo8((Wv[j0:j0 + 512] * S).T.astype(f32))
        wpb = np.ascontiguousarray(
            Wp[:, j0:j0 + 512].T.astype(f32).reshape(4, P, C)
            .transpose(1, 0, 2)).astype(BF)          # [128, 4, 1024]
        in_maps.append({
            "xh": xh, "xl": xl,
            "wqh": wqh, "wql": wql,
            "wkh": wkh, "wkl": wkl,
            "wvh": wvh, "wvl": wvl,
            "wpb": wpb,
        })
    return in_maps


def kernel(x, Wq, Wk, Wv, Wp, _trace=False):
    from concourse.bass_utils import run_bass_kernel_spmd

    x = np.asarray(x); Wq = np.asarray(Wq); Wk = np.asarray(Wk)
    Wv = np.asarray(Wv); Wp = np.asarray(Wp)

    if "nc" not in _CACHE:
        _CACHE["nc"] = _build()
    nc = _CACHE["nc"]

    in_maps = _prep_inputs(x, Wq, Wk, Wv, Wp)
    res = run_bass_kernel_spmd(nc, in_maps, core_ids=list(range(NC)),
                               trace=_trace)
    outs = [r["out"] for r in res.results]
    full = np.empty((B, T, C), np.float32)
    for b in range(B):
        full[b] = outs[2 * b] + outs[2 * b + 1]
    if _trace:
        _CACHE["last_results"] = res
    return full


# revision 6
# speedup vs baseline: 1.1322x; 1.0271x over previous
"""Causal self-attention (B=4, T=2048, C=1024, H=16) on 8 TRN2 NeuronCores — v2.

Sharding: core c -> batch b = c//2, head-group g2 = c%2 (8 heads = 4 head-pairs
g, feature columns j0 = g2*512).  Host sums the two partial out-projections
per batch.  No collectives.

Dataflow (per core), engineered against the TimelineSim cost model:
  - QKV projections: fp8e4 DoubleRow matmuls (K=256/instr, 0.5 cyc/row),
    3-term hi/lo error compensation (xh@wh + xh@wl + xl@wh); weights
    pre-scaled by S=32 on host so fp8 stays in the normal range.
  - QK logits: bf16 q/k (1 cyc/row at any N -> full diagonal narrowing).
  - exp on Act: scale=1/(S^2 sqrt(hd)) folded in, bf16 out.
  - causal staircase: gpsimd affine_select on bf16 diag tiles.
  - AV transposed: out y[q-tile, 65] with the attention weights stationary
    ([128,128] lhsT) and [v|ones] bf16 moving (65 rows/matmul); rowsums land
    in psum column 64.  One accumulation group per psum bank at a time
    (2KB zero-region discipline), normalize immediately per q-tile.
  - y back to feature-major via DMA xbar transpose (no PE/DVE involved).
  - out-projection: plain bf16 matmuls; 1/S scale folded into the out copy.
  - emission interleaving: each block's AV/normalize tail and the next
    t-block's QKV tiles drip into the Act-paced QK/exp stream so neither
    PE nor Act ever drains.
"""
import numpy as np

B, T, C = 4, 2048, 1024
NC = 8
P = 128
CO = 4           # 256-wide contraction chunks for QKV DoubleRow
QB = 512
NQB = T // QB    # 4
NKT = T // P     # 16
D = 64
W65 = 65
S = 32.0

_CACHE = {}

CFG = {"pp": 2, "attp": 2, "aep": 40, "xtp": 3, "ynp": 3, "rcpp": 3, "obp": 6}


def _build():
    from contextlib import ExitStack
    import concourse.tile as tile
    from concourse import bacc, mybir

    F32 = mybir.dt.float32
    BF16 = mybir.dt.bfloat16
    F8E4 = mybir.dt.float8e4
    AF = mybir.ActivationFunctionType
    MUL = mybir.AluOpType.mult
    DR = mybir.MatmulPerfMode.DoubleRow
    ESCALE = 1.0 / (S * S * 8.0)

    nc = bacc.Bacc("TRN2", target_bir_lowering=False, debug=False,
                   dynamic_dma_scratch_size=2048)
    dram = {}
    for nm in ("xh", "xl"):
        dram[nm] = nc.dram_tensor(nm, [P, CO, 2, T], F8E4,
                                  kind="ExternalInput").ap()
    for nm in ("wqh", "wql", "wkh", "wkl", "wvh", "wvl"):
        dram[nm] = nc.dram_tensor(nm, [P, CO, 2, QB], F8E4,
                                  kind="ExternalInput").ap()
    dram["wpb"] = nc.dram_tensor("wpb", [P, 4, C], BF16,
                                 kind="ExternalInput").ap()
    out = nc.dram_tensor("out", [T, C], F32, kind="ExternalOutput").ap()

    with tile.TileContext(nc) as tc, ExitStack() as ctx:
        persist = ctx.enter_context(tc.tile_pool(name="persist", bufs=1))
        qt = [persist.tile([P, T], BF16, tag=f"qt{g}", name=f"qt{g}")
              for g in range(4)]
        kt = [persist.tile([P, T], BF16, tag=f"kt{g}", name=f"kt{g}")
              for g in range(4)]
        # v natural: [kpos, ktile, head, 65] bf16, col 64 = ones
        vtp = persist.tile([P, NKT, 8, W65], BF16, tag="vtp", name="vtp")
        # y feature-major bf16, per q-super-block: [feat(h2*64+d), g, 512]
        yt = [persist.tile([P, 4, QB], BF16, tag=f"yt{qb}", name=f"yt{qb}")
              for qb in range(NQB)]
        wts = {}
        for nm in ("wqh", "wql", "wkh", "wkl", "wvh", "wvl"):
            wts[nm] = persist.tile([P, CO, 2, QB], F8E4, tag=nm, name=nm)
        wpb = persist.tile([P, 4, C], BF16, tag="wpb", name="wpb")

        nc.vector.memset(vtp[:, :, :, D:W65], 1.0)

        with tc.tile_pool(name="xtp", bufs=CFG["xtp"]) as xtp, \
             tc.tile_pool(name="pp", bufs=CFG["pp"], space="PSUM") as pp, \
             tc.tile_pool(name="attp", bufs=CFG["attp"], space="PSUM") as attp, \
             tc.tile_pool(name="yap", bufs=1, space="PSUM") as yap, \
             tc.tile_pool(name="aep", bufs=CFG["aep"]) as aep, \
             tc.tile_pool(name="rcpp", bufs=CFG["rcpp"]) as rcpp, \
             tc.tile_pool(name="ynp", bufs=CFG["ynp"]) as ynp, \
             tc.tile_pool(name="obp", bufs=CFG["obp"]) as obp:

            # ---- input DMAs: whole tensors, 3 queues, first-needed first
            xh0 = xtp.tile([P, CO, 2, QB], F8E4, tag="xh", name="xh0")
            xl0 = xtp.tile([P, CO, 2, QB], F8E4, tag="xl", name="xl0")
            nc.sync.dma_start(wts["wqh"][:], dram["wqh"])
            nc.gpsimd.dma_start(xh0[:], dram["xh"][:, :, :, 0:QB])
            nc.sync.dma_start(wts["wql"][:], dram["wql"])
            nc.gpsimd.dma_start(xl0[:], dram["xl"][:, :, :, 0:QB])
            nc.sync.dma_start(wts["wkh"][:], dram["wkh"])
            nc.sync.dma_start(wts["wkl"][:], dram["wkl"])
            for nm in ("wvh", "wvl"):
                nc.scalar.dma_start(wts[nm][:], dram[nm])
            nc.scalar.dma_start(wpb[:], dram["wpb"])

            xts = {0: (xh0, xl0)}

            def load_x(tb):
                if tb in xts:
                    return xts[tb]
                xh_t = xtp.tile([P, CO, 2, QB], F8E4, tag="xh", name="xh")
                xl_t = xtp.tile([P, CO, 2, QB], F8E4, tag="xl", name="xl")
                nc.sync.dma_start(
                    xh_t[:], dram["xh"][:, :, :, tb * QB:(tb + 1) * QB])
                nc.sync.dma_start(
                    xl_t[:], dram["xl"][:, :, :, tb * QB:(tb + 1) * QB])
                xts[tb] = (xh_t, xl_t)
                return xts[tb]

            def emit_qk_tile(proj, g, tb):
                xh_t, xl_t = xts[tb]
                wh, wl = wts[f"w{proj}h"], wts[f"w{proj}l"]
                dst = qt if proj == "q" else kt
                ps = pp.tile([P, QB], F32, tag="pp", name="pp")
                n = 0
                for wt_, xt_ in ((wh, xh_t), (wl, xh_t), (wh, xl_t)):
                    for ch in range(CO):
                        nc.tensor.matmul(
                            ps[:], wt_[:, ch, :, g * P:(g + 1) * P],
                            xt_[:, ch], start=(n == 0),
                            stop=(n == 3 * CO - 1), perf_mode=DR)
                        n += 1
                        if n in (4, 8):
                            yield
                nc.vector.tensor_copy(dst[g][:, tb * QB:(tb + 1) * QB], ps[:])

            def emit_v_tile(tt, tb):
                xh_t, xl_t = xts[tb]
                ki = tb * 4 + tt
                ps = pp.tile([P, QB], F32, tag="pp", name="pp")
                n = 0
                for wt_, xt_ in ((wts["wvh"], xh_t), (wts["wvl"], xh_t),
                                 (wts["wvh"], xl_t)):
                    for ch in range(CO):
                        nc.tensor.matmul(
                            ps[:], xt_[:, ch, :, tt * P:(tt + 1) * P],
                            wt_[:, ch], start=(n == 0),
                            stop=(n == 3 * CO - 1), perf_mode=DR)
                        n += 1
                        if n in (4, 8):
                            yield
                nc.vector.tensor_copy(
                    vtp[:, ki, :, 0:D],
                    ps[:].rearrange("p (h d) -> p h d", d=D))

            def emit_op_tile(tt, mh):
                po = yap.tile([P, QB], F32, tag=f"ya{(tt * 2 + mh) % 2}",
                              name="po")
                for g in range(4):
                    nc.tensor.matmul(
                        po[:], yt[tt // 4][:, g, (tt % 4) * P:(tt % 4 + 1) * P],
                        wpb[:, g, mh * QB:(mh + 1) * QB],
                        start=(g == 0), stop=(g == 3))
                    if g < 3:
                        yield
                ob = obp.tile([P, QB], F32, tag="ob", name="ob")
                nc.vector.tensor_scalar(ob[:], po[:], 1.0 / S, None, MUL)
                nc.sync.dma_start(
                    out[tt * P:(tt + 1) * P, mh * QB:(mh + 1) * QB], ob[:])

            def _unit(fn, pieces, *a):
                def mk():
                    return fn(*a)
                mk.pieces = pieces
                return mk

            def qkv_units(tb):
                return ([_unit(emit_qk_tile, 3, "q", g, tb) for g in range(4)]
                        + [_unit(emit_v_tile, 3, tt, tb) for tt in range(4)]
                        + [_unit(emit_qk_tile, 3, "k", g, tb) for g in range(4)])

            def op_units(qb):
                return [_unit(emit_op_tile, 4, tt, mh)
                        for tt in range(qb * 4, qb * 4 + 4) for mh in range(2)]

            def block_tail(g, qb, aes):
                """AV sweeps + normalize + DMA-transpose for one block."""
                rcp = rcpp.tile([P, NQB, 2], F32, tag="rcp", name="rcp")
                yn = ynp.tile([P, NQB, 2, D], BF16, tag="yn", name="yn")
                for qi in range(NQB):
                    lastk = qb * 4 + qi
                    for h2 in range(2):
                        ya = yap.tile([P, QB], F32, tag=f"ya{h2}",
                                      name=f"ya{h2}")
                        for ki in range(lastk + 1):
                            nc.tensor.matmul(
                                ya[:, 0:W65],
                                aes[ki][:, h2, qi * P:(qi + 1) * P],
                                vtp[:, ki, 2 * g + h2, :],
                                start=(ki == 0), stop=(ki == lastk))
                        nc.vector.reciprocal_approx_fast(
                            rcp[:, qi, h2:h2 + 1], ya[:, D:D + 1])
                        nc.vector.tensor_tensor(
                            yn[:, qi, h2, :], ya[:, 0:D],
                            rcp[:, qi, h2:h2 + 1].broadcast_to([P, D]), MUL)
                        yield
                    nc.sync.dma_start_transpose(
                        yt[qb][:, g, qi * P:(qi + 1) * P], yn[:, qi])

            tail_gens = []

            def advance_tail():
                while tail_gens:
                    try:
                        next(tail_gens[0])
                        return True
                    except StopIteration:
                        tail_gens.pop(0)
                return False

            # QKV for tb=0 runs unaccompanied (pipeline fill)
            for u in qkv_units(0):
                for _ in u():
                    pass

            for qb in range(NQB):
                q0 = qb * QB
                extras = []
                if qb + 1 < NQB:
                    load_x(qb + 1)
                    extras += qkv_units(qb + 1)
                if qb == 2:
                    extras += op_units(0)
                elif qb == 3:
                    extras += op_units(1) + op_units(2)
                gens = [u() for u in extras]
                steps = 4 * (qb * 4 + 4)
                acc = 0.0
                per_step = sum(u.pieces for u in extras) / steps

                step_i = [0]

                def drip():
                    nonlocal acc
                    step_i[0] += 1
                    acc += per_step * 1.5 * step_i[0] / max(steps, 1)
                    while acc >= 1.0 and gens:
                        try:
                            next(gens[0])
                        except StopIteration:
                            gens.pop(0)
                        else:
                            acc -= 1.0

                for g in range(4):
                    nk = qb * 4 + 4
                    aes = []
                    for ki in range(nk):
                        r = ki - qb * 4  # >=0 on diagonal tiles
                        dq = r * P if r >= 0 else 0
                        ap = attp.tile([P, 2, QB], F32, tag="att", name="att")
                        for h2 in range(2):
                            rows = slice(h2 * D, h2 * D + D)
                            nc.tensor.matmul(
                                ap[:, h2, dq:QB],
                                kt[g][rows, ki * P:(ki + 1) * P],
                                qt[g][rows, q0 + dq:q0 + QB],
                                start=True, stop=True)
                        ae = aep.tile([P, 2, QB], BF16, tag="ae", name="ae")
                        nc.scalar.activation(ae[:, :, dq:QB], ap[:, :, dq:QB],
                                             AF.Exp, scale=ESCALE)
                        if r >= 0:
                            for h2 in range(2):
                                nc.gpsimd.affine_select(
                                    out=ae[:, h2, dq:dq + P],
                                    in_=ae[:, h2, dq:dq + P],
                                    compare_op=mybir.AluOpType.is_ge,
                                    fill=0.0, base=0,
                                    pattern=[[1, P]], channel_multiplier=-1)
                        aes.append(ae)
                        advance_tail()
                        advance_tail()
                        drip()
                    tail_gens.append(block_tail(g, qb, aes))
                while gens:
                    try:
                        next(gens[0])
                    except StopIteration:
                        gens.pop(0)
            while advance_tail():
                pass
            # final out-projection block
            for u in op_units(NQB - 1):
                for _ in u():
                    pass

    nc.finalize()
    return nc


def _prep_inputs(x, Wq, Wk, Wv, Wp):
    import ml_dtypes
    F8 = ml_dtypes.float8_e4m3fn
    BF = ml_dtypes.bfloat16
    f32 = np.float32

    def dr_layout(a):  # [1024, N] -> [128, 4, 2, N] (ci, ch, j)
        n = a.shape[1]
        return np.ascontiguousarray(
            a.reshape(CO, 2, P, n).transpose(2, 0, 1, 3))

    def hilo8(a):
        h = np.clip(a, -448, 448).astype(F8)
        l = np.clip(a - h.astype(f32), -448, 448).astype(F8)
        return dr_layout(h), dr_layout(l)

    in_maps = []
    for c in range(NC):
        b, g2 = c // 2, c % 2
        j0 = g2 * 512
        xh, xl = hilo8(x[b].T.astype(f32))
        wqh, wql = hilo8((Wq[j0:j0 + 512] * S).T.astype(f32))
        wkh, wkl = hilo8((Wk[j0:j0 + 512] * S).T.astype(f32))
        wvh, wvl = hilo8((Wv[j0:j0 + 512] * S).T.astype(f32))
        wpb = np.ascontiguousarray(
            Wp[:, j0:j0 + 512].T.astype(f32).reshape(4, P, C)
            .transpose(1, 0, 2)).astype(BF)          # [128, 4, 1024]
        in_maps.append({
            "xh": xh, "xl": xl,
            "wqh": wqh, "wql": wql,
            "wkh": wkh, "wkl": wkl,
            "wvh": wvh, "wvl": wvl,
            "wpb": wpb,
        })
    return in_maps


def kernel(x, Wq, Wk, Wv, Wp, _trace=False):
    from concourse.bass_utils import run_bass_kernel_spmd

    x = np.asarray(x); Wq = np.asarray(Wq); Wk = np.asarray(Wk)
    Wv = np.asarray(Wv); Wp = np.asarray(Wp)

    if "nc" not in _CACHE:
        _CACHE["nc"] = _build()
    nc = _CACHE["nc"]

    in_maps = _prep_inputs(x, Wq, Wk, Wv, Wp)
    res = run_bass_kernel_spmd(nc, in_maps, core_ids=list(range(NC)),
                               trace=_trace)
    outs = [r["out"] for r in res.results]
    full = np.empty((B, T, C), np.float32)
    for b in range(B):
        full[b] = outs[2 * b] + outs[2 * b + 1]
    if _trace:
        _CACHE["last_results"] = res
    return full


# revision 7
# speedup vs baseline: 1.1473x; 1.0134x over previous
"""Causal self-attention (B=4, T=2048, C=1024, H=16) on 8 TRN2 NeuronCores — v2.

Sharding: core c -> batch b = c//2, head-group g2 = c%2 (8 heads = 4 head-pairs
g, feature columns j0 = g2*512).  Host sums the two partial out-projections
per batch.  No collectives.

Dataflow (per core), engineered against the TimelineSim cost model:
  - QKV projections: fp8e4 DoubleRow matmuls (K=256/instr, 0.5 cyc/row),
    3-term hi/lo error compensation (xh@wh + xh@wl + xl@wh); weights
    pre-scaled by S=32 on host so fp8 stays in the normal range.
  - QK logits: bf16 q/k (1 cyc/row at any N -> full diagonal narrowing).
  - exp on Act: scale=1/(S^2 sqrt(hd)) folded in, bf16 out.
  - causal staircase: gpsimd affine_select on bf16 diag tiles.
  - AV transposed: out y[q-tile, 65] with the attention weights stationary
    ([128,128] lhsT) and [v|ones] bf16 moving (65 rows/matmul); rowsums land
    in psum column 64.  One accumulation group per psum bank at a time
    (2KB zero-region discipline), normalize immediately per q-tile.
  - y back to feature-major via DMA xbar transpose (no PE/DVE involved).
  - out-projection: plain bf16 matmuls; 1/S scale folded into the out copy.
  - emission interleaving: each block's AV/normalize tail and the next
    t-block's QKV tiles drip into the Act-paced QK/exp stream so neither
    PE nor Act ever drains.
"""
import numpy as np

B, T, C = 4, 2048, 1024
NC = 8
P = 128
CO = 4           # 256-wide contraction chunks for QKV DoubleRow
QB = 512
NQB = T // QB    # 4
NKT = T // P     # 16
D = 64
W65 = 65
S = 32.0

_CACHE = {}

CFG = {"pp": 2, "attp": 2, "aep": 40, "xtp": 3, "ynp": 3, "rcpp": 3, "obp": 6}


def _build():
    from contextlib import ExitStack
    import concourse.tile as tile
    from concourse import bacc, mybir

    F32 = mybir.dt.float32
    BF16 = mybir.dt.bfloat16
    F8E4 = mybir.dt.float8e4
    AF = mybir.ActivationFunctionType
    MUL = mybir.AluOpType.mult
    DR = mybir.MatmulPerfMode.DoubleRow
    ESCALE = 1.0 / (S * S * 8.0)

    nc = bacc.Bacc("TRN2", target_bir_lowering=False, debug=False,
                   dynamic_dma_scratch_size=2048)
    dram = {}
    for nm in ("xh", "xl"):
        dram[nm] = nc.dram_tensor(nm, [P, CO, 2, T], F8E4,
                                  kind="ExternalInput").ap()
    for nm in ("wqh", "wql", "wkh", "wkl", "wvh", "wvl"):
        dram[nm] = nc.dram_tensor(nm, [P, CO, 2, QB], F8E4,
                                  kind="ExternalInput").ap()
    dram["wpb"] = nc.dram_tensor("wpb", [P, 4, C], BF16,
                                 kind="ExternalInput").ap()
    out = nc.dram_tensor("out", [T, C], F32, kind="ExternalOutput").ap()

    with tile.TileContext(nc) as tc, ExitStack() as ctx:
        persist = ctx.enter_context(tc.tile_pool(name="persist", bufs=1))
        qt = [persist.tile([P, T], BF16, tag=f"qt{g}", name=f"qt{g}")
              for g in range(4)]
        kt = [persist.tile([P, T], BF16, tag=f"kt{g}", name=f"kt{g}")
              for g in range(4)]
        # v natural: [kpos, ktile, head, 65] bf16, col 64 = ones
        vtp = persist.tile([P, NKT, 8, W65], BF16, tag="vtp", name="vtp")
        # y feature-major bf16, per q-super-block: [feat(h2*64+d), g, 512]
        yt = [persist.tile([P, 4, QB], BF16, tag=f"yt{qb}", name=f"yt{qb}")
              for qb in range(NQB)]
        wts = {}
        for nm in ("wqh", "wql", "wkh", "wkl", "wvh", "wvl"):
            wts[nm] = persist.tile([P, CO, 2, QB], F8E4, tag=nm, name=nm)
        wpb = persist.tile([P, 4, C], BF16, tag="wpb", name="wpb")

        nc.vector.memset(vtp[:, :, :, D:W65], 1.0)

        with tc.tile_pool(name="xtp", bufs=CFG["xtp"]) as xtp, \
             tc.tile_pool(name="pp", bufs=CFG["pp"], space="PSUM") as pp, \
             tc.tile_pool(name="attp", bufs=CFG["attp"], space="PSUM") as attp, \
             tc.tile_pool(name="yap", bufs=1, space="PSUM") as yap, \
             tc.tile_pool(name="aep", bufs=CFG["aep"]) as aep, \
             tc.tile_pool(name="rcpp", bufs=CFG["rcpp"]) as rcpp, \
             tc.tile_pool(name="ynp", bufs=CFG["ynp"]) as ynp, \
             tc.tile_pool(name="obp", bufs=CFG["obp"]) as obp:

            # ---- input DMAs: whole tensors, 3 queues, first-needed first
            xh0 = xtp.tile([P, CO, 2, QB], F8E4, tag="xh", name="xh0")
            xl0 = xtp.tile([P, CO, 2, QB], F8E4, tag="xl", name="xl0")
            nc.sync.dma_start(wts["wqh"][:], dram["wqh"])
            nc.gpsimd.dma_start(xh0[:], dram["xh"][:, :, :, 0:QB])
            nc.sync.dma_start(wts["wql"][:], dram["wql"])
            nc.gpsimd.dma_start(xl0[:], dram["xl"][:, :, :, 0:QB])
            nc.sync.dma_start(wts["wkh"][:], dram["wkh"])
            nc.sync.dma_start(wts["wkl"][:], dram["wkl"])
            for nm in ("wvh", "wvl"):
                nc.scalar.dma_start(wts[nm][:], dram[nm])
            nc.scalar.dma_start(wpb[:], dram["wpb"])

            xts = {0: (xh0, xl0)}

            def load_x(tb):
                if tb in xts:
                    return xts[tb]
                xh_t = xtp.tile([P, CO, 2, QB], F8E4, tag="xh", name="xh")
                xl_t = xtp.tile([P, CO, 2, QB], F8E4, tag="xl", name="xl")
                nc.sync.dma_start(
                    xh_t[:], dram["xh"][:, :, :, tb * QB:(tb + 1) * QB])
                nc.sync.dma_start(
                    xl_t[:], dram["xl"][:, :, :, tb * QB:(tb + 1) * QB])
                xts[tb] = (xh_t, xl_t)
                return xts[tb]

            def emit_qk_tile(proj, g, tb):
                xh_t, xl_t = xts[tb]
                wh, wl = wts[f"w{proj}h"], wts[f"w{proj}l"]
                dst = qt if proj == "q" else kt
                ps = pp.tile([P, QB], F32, tag="pp", name="pp")
                n = 0
                for wt_, xt_ in ((wh, xh_t), (wl, xh_t), (wh, xl_t)):
                    for ch in range(CO):
                        nc.tensor.matmul(
                            ps[:], wt_[:, ch, :, g * P:(g + 1) * P],
                            xt_[:, ch], start=(n == 0),
                            stop=(n == 3 * CO - 1), perf_mode=DR)
                        n += 1
                        if n in (4, 8):
                            yield
                nc.vector.tensor_copy(dst[g][:, tb * QB:(tb + 1) * QB], ps[:])

            def emit_v_tile(tt, tb):
                xh_t, xl_t = xts[tb]
                ki = tb * 4 + tt
                ps = pp.tile([P, QB], F32, tag="pp", name="pp")
                n = 0
                for wt_, xt_ in ((wts["wvh"], xh_t), (wts["wvl"], xh_t),
                                 (wts["wvh"], xl_t)):
                    for ch in range(CO):
                        nc.tensor.matmul(
                            ps[:], xt_[:, ch, :, tt * P:(tt + 1) * P],
                            wt_[:, ch], start=(n == 0),
                            stop=(n == 3 * CO - 1), perf_mode=DR)
                        n += 1
                        if n in (4, 8):
                            yield
                nc.vector.tensor_copy(
                    vtp[:, ki, :, 0:D],
                    ps[:].rearrange("p (h d) -> p h d", d=D))

            def emit_op_tile(tt, mh):
                po = yap.tile([P, QB], F32, tag=f"ya{(tt * 2 + mh) % 2}",
                              name="po")
                for g in range(4):
                    nc.tensor.matmul(
                        po[:], yt[tt // 4][:, g, (tt % 4) * P:(tt % 4 + 1) * P],
                        wpb[:, g, mh * QB:(mh + 1) * QB],
                        start=(g == 0), stop=(g == 3))
                    if g < 3:
                        yield
                ob = obp.tile([P, QB], F32, tag="ob", name="ob")
                nc.vector.tensor_scalar(ob[:], po[:], 1.0 / S, None, MUL)
                nc.sync.dma_start(
                    out[tt * P:(tt + 1) * P, mh * QB:(mh + 1) * QB], ob[:])

            def _unit(fn, pieces, *a):
                def mk():
                    return fn(*a)
                mk.pieces = pieces
                return mk

            def qkv_units(tb):
                return ([_unit(emit_qk_tile, 3, "q", g, tb) for g in range(4)]
                        + [_unit(emit_v_tile, 3, tt, tb) for tt in range(4)]
                        + [_unit(emit_qk_tile, 3, "k", g, tb) for g in range(4)])

            def op_units(qb):
                return [_unit(emit_op_tile, 4, tt, mh)
                        for tt in range(qb * 4, qb * 4 + 4) for mh in range(2)]

            def block_tail(g, qb, aes):
                """AV sweeps + normalize + DMA-transpose for one block."""
                rcp = rcpp.tile([P, NQB, 2], F32, tag="rcp", name="rcp")
                yn = ynp.tile([P, NQB, 2, D], BF16, tag="yn", name="yn")
                for qi in range(NQB):
                    lastk = qb * 4 + qi
                    for h2 in range(2):
                        ya = yap.tile([P, QB], F32, tag=f"ya{h2}",
                                      name=f"ya{h2}")
                        for ki in range(lastk + 1):
                            nc.tensor.matmul(
                                ya[:, 0:W65],
                                aes[ki][:, h2, qi * P:(qi + 1) * P],
                                vtp[:, ki, 2 * g + h2, :],
                                start=(ki == 0), stop=(ki == lastk))
                        nc.vector.reciprocal_approx_fast(
                            rcp[:, qi, h2:h2 + 1], ya[:, D:D + 1])
                        nc.vector.tensor_tensor(
                            yn[:, qi, h2, :], ya[:, 0:D],
                            rcp[:, qi, h2:h2 + 1].broadcast_to([P, D]), MUL)
                        yield
                    nc.sync.dma_start_transpose(
                        yt[qb][:, g, qi * P:(qi + 1) * P], yn[:, qi])

            tail_gens = []

            def advance_tail():
                while tail_gens:
                    try:
                        next(tail_gens[0])
                        return True
                    except StopIteration:
                        tail_gens.pop(0)
                return False

            # QKV for tb=0 runs unaccompanied (pipeline fill)
            for u in qkv_units(0):
                for _ in u():
                    pass

            for qb in range(NQB):
                q0 = qb * QB
                extras = []
                if qb + 1 < NQB:
                    load_x(qb + 1)
                    extras += qkv_units(qb + 1)
                if qb == 2:
                    extras += op_units(0)
                elif qb == 3:
                    extras += op_units(1) + op_units(2)
                gens = [u() for u in extras]
                steps = 4 * (qb * 4 + 4)
                acc = 0.0
                per_step = sum(u.pieces for u in extras) / steps

                step_i = [0]

                def drip():
                    nonlocal acc
                    step_i[0] += 1
                    acc += per_step * 1.4 * step_i[0] / max(steps, 1)
                    while acc >= 1.0 and gens:
                        try:
                            next(gens[0])
                        except StopIteration:
                            gens.pop(0)
                        else:
                            acc -= 1.0

                for g in range(4):
                    nk = qb * 4 + 4
                    aes = []
                    for ki in range(nk):
                        r = ki - qb * 4  # >=0 on diagonal tiles
                        dq = r * P if r >= 0 else 0
                        ap = attp.tile([P, 2, QB], F32, tag="att", name="att")
                        for h2 in range(2):
                            rows = slice(h2 * D, h2 * D + D)
                            nc.tensor.matmul(
                                ap[:, h2, dq:QB],
                                kt[g][rows, ki * P:(ki + 1) * P],
                                qt[g][rows, q0 + dq:q0 + QB],
                                start=True, stop=True)
                        ae = aep.tile([P, 2, QB], BF16, tag="ae", name="ae")
                        nc.scalar.activation(ae[:, :, dq:QB], ap[:, :, dq:QB],
                                             AF.Exp, scale=ESCALE)
                        if r >= 0:
                            for h2 in range(2):
                                nc.gpsimd.affine_select(
                                    out=ae[:, h2, dq:dq + P],
                                    in_=ae[:, h2, dq:dq + P],
                                    compare_op=mybir.AluOpType.is_ge,
                                    fill=0.0, base=0,
                                    pattern=[[1, P]], channel_multiplier=-1)
                        aes.append(ae)
                        advance_tail()
                        advance_tail()
                        if len(tail_gens) > 1:
                            advance_tail()
                        drip()
                    tail_gens.append(block_tail(g, qb, aes))
                while gens:
                    try:
                        next(gens[0])
                    except StopIteration:
                        gens.pop(0)
            while advance_tail():
                pass
            # final out-projection block
            for u in op_units(NQB - 1):
                for _ in u():
                    pass

    nc.finalize()
    return nc


def _prep_inputs(x, Wq, Wk, Wv, Wp):
    import ml_dtypes
    F8 = ml_dtypes.float8_e4m3fn
    BF = ml_dtypes.bfloat16
    f32 = np.float32

    def dr_layout(a):  # [1024, N] -> [128, 4, 2, N] (ci, ch, j)
        n = a.shape[1]
        return np.ascontiguousarray(
            a.reshape(CO, 2, P, n).transpose(2, 0, 1, 3))

    def hilo8(a):
        h = np.clip(a, -448, 448).astype(F8)
        l = np.clip(a - h.astype(f32), -448, 448).astype(F8)
        return dr_layout(h), dr_layout(l)

    in_maps = []
    for c in range(NC):
        b, g2 = c // 2, c % 2
        j0 = g2 * 512
        xh, xl = hilo8(x[b].T.astype(f32))
        wqh, wql = hilo8((Wq[j0:j0 + 512] * S).T.astype(f32))
        wkh, wkl = hilo8((Wk[j0:j0 + 512] * S).T.astype(f32))
        wvh, wvl = hilo8((Wv[j0:j0 + 512] * S).T.astype(f32))
        wpb = np.ascontiguousarray(
            Wp[:, j0:j0 + 512].T.astype(f32).reshape(4, P, C)
            .transpose(1, 0, 2)).astype(BF)          # [128, 4, 1024]
        in_maps.append({
            "xh": xh, "xl": xl,
            "wqh": wqh, "wql": wql,
            "wkh": wkh, "wkl": wkl,
            "wvh": wvh, "wvl": wvl,
            "wpb": wpb,
        })
    return in_maps


def kernel(x, Wq, Wk, Wv, Wp, _trace=False):
    from concourse.bass_utils import run_bass_kernel_spmd

    x = np.asarray(x); Wq = np.asarray(Wq); Wk = np.asarray(Wk)
    Wv = np.asarray(Wv); Wp = np.asarray(Wp)

    if "nc" not in _CACHE:
        _CACHE["nc"] = _build()
    nc = _CACHE["nc"]

    in_maps = _prep_inputs(x, Wq, Wk, Wv, Wp)
    res = run_bass_kernel_spmd(nc, in_maps, core_ids=list(range(NC)),
                               trace=_trace)
    outs = [r["out"] for r in res.results]
    full = np.empty((B, T, C), np.float32)
    for b in range(B):
        full[b] = outs[2 * b] + outs[2 * b + 1]
    if _trace:
        _CACHE["last_results"] = res
    return full


# revision 8
# speedup vs baseline: 1.1505x; 1.0028x over previous
"""Causal self-attention (B=4, T=2048, C=1024, H=16) on 8 TRN2 NeuronCores — v2.

Sharding: core c -> batch b = c//2, head-group g2 = c%2 (8 heads = 4 head-pairs
g, feature columns j0 = g2*512).  Host sums the two partial out-projections
per batch.  No collectives.

Dataflow (per core), engineered against the TimelineSim cost model:
  - QKV projections: fp8e4 DoubleRow matmuls (K=256/instr, 0.5 cyc/row),
    3-term hi/lo error compensation (xh@wh + xh@wl + xl@wh); weights
    pre-scaled by S=32 on host so fp8 stays in the normal range.
  - QK logits: bf16 q/k (1 cyc/row at any N -> full diagonal narrowing).
  - exp on Act: scale=1/(S^2 sqrt(hd)) folded in, bf16 out.
  - causal staircase: gpsimd affine_select on bf16 diag tiles.
  - AV transposed: out y[q-tile, 65] with the attention weights stationary
    ([128,128] lhsT) and [v|ones] bf16 moving (65 rows/matmul); rowsums land
    in psum column 64.  One accumulation group per psum bank at a time
    (2KB zero-region discipline), normalize immediately per q-tile.
  - y back to feature-major via DMA xbar transpose (no PE/DVE involved).
  - out-projection: plain bf16 matmuls; 1/S scale folded into the out copy.
  - emission interleaving: each block's AV/normalize tail and the next
    t-block's QKV tiles drip into the Act-paced QK/exp stream so neither
    PE nor Act ever drains.
"""
import numpy as np

B, T, C = 4, 2048, 1024
NC = 8
P = 128
CO = 4           # 256-wide contraction chunks for QKV DoubleRow
QB = 512
NQB = T // QB    # 4
NKT = T // P     # 16
D = 64
W65 = 65
S = 32.0

_CACHE = {}

CFG = {"pp": 2, "attp": 2, "aep": 40, "xtp": 3, "ynp": 3, "rcpp": 3, "obp": 6}


def _build():
    from contextlib import ExitStack
    import concourse.tile as tile
    from concourse import bacc, mybir

    F32 = mybir.dt.float32
    BF16 = mybir.dt.bfloat16
    F8E4 = mybir.dt.float8e4
    AF = mybir.ActivationFunctionType
    MUL = mybir.AluOpType.mult
    DR = mybir.MatmulPerfMode.DoubleRow
    ESCALE = 1.0 / (S * S * 8.0)

    nc = bacc.Bacc("TRN2", target_bir_lowering=False, debug=False,
                   dynamic_dma_scratch_size=2048)
    dram = {}
    for nm in ("xh", "xl"):
        dram[nm] = nc.dram_tensor(nm, [P, CO, 2, T], F8E4,
                                  kind="ExternalInput").ap()
    for nm in ("wqh", "wql", "wkh", "wkl", "wvh", "wvl"):
        dram[nm] = nc.dram_tensor(nm, [P, CO, 2, QB], F8E4,
                                  kind="ExternalInput").ap()
    dram["wpb"] = nc.dram_tensor("wpb", [P, 4, C], BF16,
                                 kind="ExternalInput").ap()
    out = nc.dram_tensor("out", [T, C], F32, kind="ExternalOutput").ap()

    with tile.TileContext(nc) as tc, ExitStack() as ctx:
        persist = ctx.enter_context(tc.tile_pool(name="persist", bufs=1))
        qt = [persist.tile([P, T], BF16, tag=f"qt{g}", name=f"qt{g}")
              for g in range(4)]
        kt = [persist.tile([P, T], BF16, tag=f"kt{g}", name=f"kt{g}")
              for g in range(4)]
        # v natural: [kpos, ktile, head, 65] bf16, col 64 = ones
        vtp = persist.tile([P, NKT, 8, W65], BF16, tag="vtp", name="vtp")
        # y feature-major bf16, per q-super-block: [feat(h2*64+d), g, 512]
        yt = [persist.tile([P, 4, QB], BF16, tag=f"yt{qb}", name=f"yt{qb}")
              for qb in range(NQB)]
        wts = {}
        for nm in ("wqh", "wql", "wkh", "wkl", "wvh", "wvl"):
            wts[nm] = persist.tile([P, CO, 2, QB], F8E4, tag=nm, name=nm)
        wpb = persist.tile([P, 4, C], BF16, tag="wpb", name="wpb")

        nc.vector.memset(vtp[:, :, :, D:W65], 1.0)

        with tc.tile_pool(name="xtp", bufs=CFG["xtp"]) as xtp, \
             tc.tile_pool(name="pp", bufs=CFG["pp"], space="PSUM") as pp, \
             tc.tile_pool(name="attp", bufs=CFG["attp"], space="PSUM") as attp, \
             tc.tile_pool(name="yap", bufs=1, space="PSUM") as yap, \
             tc.tile_pool(name="aep", bufs=CFG["aep"]) as aep, \
             tc.tile_pool(name="rcpp", bufs=CFG["rcpp"]) as rcpp, \
             tc.tile_pool(name="ynp", bufs=CFG["ynp"]) as ynp, \
             tc.tile_pool(name="obp", bufs=CFG["obp"]) as obp:

            # ---- input DMAs: whole tensors, 3 queues, first-needed first
            xh0 = xtp.tile([P, CO, 2, QB], F8E4, tag="xh", name="xh0")
            xl0 = xtp.tile([P, CO, 2, QB], F8E4, tag="xl", name="xl0")
            nc.sync.dma_start(wts["wqh"][:], dram["wqh"])
            nc.gpsimd.dma_start(xh0[:], dram["xh"][:, :, :, 0:QB])
            nc.sync.dma_start(wts["wql"][:], dram["wql"])
            nc.gpsimd.dma_start(xl0[:], dram["xl"][:, :, :, 0:QB])
            nc.sync.dma_start(wts["wkh"][:], dram["wkh"])
            nc.sync.dma_start(wts["wkl"][:], dram["wkl"])
            for nm in ("wvh", "wvl"):
                nc.scalar.dma_start(wts[nm][:], dram[nm])
            nc.scalar.dma_start(wpb[:], dram["wpb"])

            xts = {0: (xh0, xl0)}

            def load_x(tb):
                if tb in xts:
                    return xts[tb]
                xh_t = xtp.tile([P, CO, 2, QB], F8E4, tag="xh", name="xh")
                xl_t = xtp.tile([P, CO, 2, QB], F8E4, tag="xl", name="xl")
                nc.sync.dma_start(
                    xh_t[:], dram["xh"][:, :, :, tb * QB:(tb + 1) * QB])
                nc.sync.dma_start(
                    xl_t[:], dram["xl"][:, :, :, tb * QB:(tb + 1) * QB])
                xts[tb] = (xh_t, xl_t)
                return xts[tb]

            def emit_qk_tile(proj, g, tb):
                xh_t, xl_t = xts[tb]
                wh, wl = wts[f"w{proj}h"], wts[f"w{proj}l"]
                dst = qt if proj == "q" else kt
                ps = pp.tile([P, QB], F32, tag="pp", name="pp")
                n = 0
                for wt_, xt_ in ((wh, xh_t), (wl, xh_t), (wh, xl_t)):
                    for ch in range(CO):
                        nc.tensor.matmul(
                            ps[:], wt_[:, ch, :, g * P:(g + 1) * P],
                            xt_[:, ch], start=(n == 0),
                            stop=(n == 3 * CO - 1), perf_mode=DR)
                        n += 1
                        if n in (4, 8):
                            yield
                nc.vector.tensor_copy(dst[g][:, tb * QB:(tb + 1) * QB], ps[:])

            def emit_v_tile(tt, tb):
                xh_t, xl_t = xts[tb]
                ki = tb * 4 + tt
                ps = pp.tile([P, QB], F32, tag="pp", name="pp")
                n = 0
                for wt_, xt_ in ((wts["wvh"], xh_t), (wts["wvl"], xh_t),
                                 (wts["wvh"], xl_t)):
                    for ch in range(CO):
                        nc.tensor.matmul(
                            ps[:], xt_[:, ch, :, tt * P:(tt + 1) * P],
                            wt_[:, ch], start=(n == 0),
                            stop=(n == 3 * CO - 1), perf_mode=DR)
                        n += 1
                        if n in (4, 8):
                            yield
                nc.vector.tensor_copy(
                    vtp[:, ki, :, 0:D],
                    ps[:].rearrange("p (h d) -> p h d", d=D))

            def emit_op_tile(tt, mh):
                po = yap.tile([P, QB], F32, tag=f"ya{(tt * 2 + mh) % 2}",
                              name="po")
                for g in range(4):
                    nc.tensor.matmul(
                        po[:], yt[tt // 4][:, g, (tt % 4) * P:(tt % 4 + 1) * P],
                        wpb[:, g, mh * QB:(mh + 1) * QB],
                        start=(g == 0), stop=(g == 3))
                    if g < 3:
                        yield
                ob = obp.tile([P, QB], F32, tag="ob", name="ob")
                nc.vector.tensor_scalar(ob[:], po[:], 1.0 / S, None, MUL)
                nc.sync.dma_start(
                    out[tt * P:(tt + 1) * P, mh * QB:(mh + 1) * QB], ob[:])

            def _unit(fn, pieces, *a):
                def mk():
                    return fn(*a)
                mk.pieces = pieces
                return mk

            def qkv_units(tb):
                return ([_unit(emit_qk_tile, 3, "q", g, tb) for g in range(4)]
                        + [_unit(emit_v_tile, 3, tt, tb) for tt in range(4)]
                        + [_unit(emit_qk_tile, 3, "k", g, tb) for g in range(4)])

            def op_units(qb):
                return [_unit(emit_op_tile, 4, tt, mh)
                        for tt in range(qb * 4, qb * 4 + 4) for mh in range(2)]

            def block_tail(g, qb, aes):
                """AV sweeps + normalize + DMA-transpose for one block."""
                rcp = rcpp.tile([P, NQB, 2], F32, tag="rcp", name="rcp")
                yn = ynp.tile([P, NQB, 2, D], BF16, tag="yn", name="yn")
                for qi in range(NQB):
                    lastk = qb * 4 + qi
                    for h2 in range(2):
                        ya = yap.tile([P, QB], F32, tag=f"ya{h2}",
                                      name=f"ya{h2}")
                        for ki in range(lastk + 1):
                            nc.tensor.matmul(
                                ya[:, 0:W65],
                                aes[ki][:, h2, qi * P:(qi + 1) * P],
                                vtp[:, ki, 2 * g + h2, :],
                                start=(ki == 0), stop=(ki == lastk))
                        nc.vector.reciprocal_approx_fast(
                            rcp[:, qi, h2:h2 + 1], ya[:, D:D + 1])
                        nc.vector.tensor_tensor(
                            yn[:, qi, h2, :], ya[:, 0:D],
                            rcp[:, qi, h2:h2 + 1].broadcast_to([P, D]), MUL)
                        yield
                    nc.sync.dma_start_transpose(
                        yt[qb][:, g, qi * P:(qi + 1) * P], yn[:, qi])

            tail_gens = []

            def advance_tail():
                while tail_gens:
                    try:
                        next(tail_gens[0])
                        return True
                    except StopIteration:
                        tail_gens.pop(0)
                return False

            # QKV for tb=0 runs unaccompanied (pipeline fill)
            for u in qkv_units(0):
                for _ in u():
                    pass

            for qb in range(NQB):
                q0 = qb * QB
                extras = []
                if qb + 1 < NQB:
                    load_x(qb + 1)
                    extras += qkv_units(qb + 1)
                if qb == 2:
                    extras += op_units(0)
                elif qb == 3:
                    extras += op_units(1) + op_units(2)
                gens = [u() for u in extras]
                steps = 4 * (qb * 4 + 4)
                acc = 0.0
                per_step = sum(u.pieces for u in extras) / steps

                step_i = [0]

                def drip():
                    nonlocal acc
                    step_i[0] += 1
                    acc += per_step * (0.8, 1.3, 1.4, 1.6)[qb] * step_i[0] / max(steps, 1)
                    while acc >= 1.0 and gens:
                        try:
                            next(gens[0])
                        except StopIteration:
                            gens.pop(0)
                        else:
                            acc -= 1.0

                for g in range(4):
                    nk = qb * 4 + 4
                    aes = []
                    for ki in range(nk):
                        r = ki - qb * 4  # >=0 on diagonal tiles
                        dq = r * P if r >= 0 else 0
                        ap = attp.tile([P, 2, QB], F32, tag="att", name="att")
                        for h2 in range(2):
                            rows = slice(h2 * D, h2 * D + D)
                            nc.tensor.matmul(
                                ap[:, h2, dq:QB],
                                kt[g][rows, ki * P:(ki + 1) * P],
                                qt[g][rows, q0 + dq:q0 + QB],
                                start=True, stop=True)
                        ae = aep.tile([P, 2, QB], BF16, tag="ae", name="ae")
                        nc.scalar.activation(ae[:, :, dq:QB], ap[:, :, dq:QB],
                                             AF.Exp, scale=ESCALE)
                        if r >= 0:
                            for h2 in range(2):
                                nc.gpsimd.affine_select(
                                    out=ae[:, h2, dq:dq + P],
                                    in_=ae[:, h2, dq:dq + P],
                                    compare_op=mybir.AluOpType.is_ge,
                                    fill=0.0, base=0,
                                    pattern=[[1, P]], channel_multiplier=-1)
                        aes.append(ae)
                        advance_tail()
                        advance_tail()
                        if len(tail_gens) > 1:
                            advance_tail()
                        drip()
                    tail_gens.append(block_tail(g, qb, aes))
                while gens:
                    try:
                        next(gens[0])
                    except StopIteration:
                        gens.pop(0)
            while advance_tail():
                pass
            # final out-projection block
            for u in op_units(NQB - 1):
                for _ in u():
                    pass

    nc.finalize()
    return nc


def _prep_inputs(x, Wq, Wk, Wv, Wp):
    import ml_dtypes
    F8 = ml_dtypes.float8_e4m3fn
    BF = ml_dtypes.bfloat16
    f32 = np.float32

    def dr_layout(a):  # [1024, N] -> [128, 4, 2, N] (ci, ch, j)
        n = a.shape[1]
        return np.ascontiguousarray(
            a.reshape(CO, 2, P, n).transpose(2, 0, 1, 3))

    def hilo8(a):
        h = np.clip(a, -448, 448).astype(F8)
        l = np.clip(a - h.astype(f32), -448, 448).astype(F8)
        return dr_layout(h), dr_layout(l)

    in_maps = []
    for c in range(NC):
        b, g2 = c // 2, c % 2
        j0 = g2 * 512
        xh, xl = hilo8(x[b].T.astype(f32))
        wqh, wql = hilo8((Wq[j0:j0 + 512] * S).T.astype(f32))
        wkh, wkl = hilo8((Wk[j0:j0 + 512] * S).T.astype(f32))
        wvh, wvl = hilo8((Wv[j0:j0 + 512] * S).T.astype(f32))
        wpb = np.ascontiguousarray(
            Wp[:, j0:j0 + 512].T.astype(f32).reshape(4, P, C)
            .transpose(1, 0, 2)).astype(BF)          # [128, 4, 1024]
        in_maps.append({
            "xh": xh, "xl": xl,
            "wqh": wqh, "wql": wql,
            "wkh": wkh, "wkl": wkl,
            "wvh": wvh, "wvl": wvl,
            "wpb": wpb,
        })
    return in_maps


def kernel(x, Wq, Wk, Wv, Wp, _trace=False):
    from concourse.bass_utils import run_bass_kernel_spmd

    x = np.asarray(x); Wq = np.asarray(Wq); Wk = np.asarray(Wk)
    Wv = np.asarray(Wv); Wp = np.asarray(Wp)

    if "nc" not in _CACHE:
        _CACHE["nc"] = _build()
    nc = _CACHE["nc"]

    in_maps = _prep_inputs(x, Wq, Wk, Wv, Wp)
    res = run_bass_kernel_spmd(nc, in_maps, core_ids=list(range(NC)),
                               trace=_trace)
    outs = [r["out"] for r in res.results]
    full = np.empty((B, T, C), np.float32)
    for b in range(B):
        full[b] = outs[2 * b] + outs[2 * b + 1]
    if _trace:
        _CACHE["last_results"] = res
    return full
